# revision 6
# baseline (speedup 1.0000x reference)
"""Trainium2 Bass kernel for ChannelAttentionModel (segment avg/max -> tiny MLP ->
sigmoid gate -> per-point scale), SPMD across 8 NeuronCores.

Sharding: batch_ids is sorted with B=16 segments; core k owns batches 2k and
2k+1 (whole batches per device). Each batch range is padded to a fixed R points
by replicating the first row of the batch (max-safe); the extra rows' sum
contribution is subtracted via a host-computed correction term.
"""

import sys

for _p in ("/opt/trn_rl_repo", "/root/.axon_site/_ro/trn_rl_repo"):
    if _p not in sys.path:
        sys.path.append(_p)

import numpy as np

import concourse.bacc as bacc
import concourse.tile as tile
from concourse import bass, mybir
from concourse.bass_utils import run_bass_kernel_spmd
from concourse.masks import make_identity

NCORES = 8
B = 16
C = 64
H = 32
RPC = 2  # batch ranges per core
TP = 4096  # points per tile
FA = TP // 128  # free-dim point groups per partition (32)
F = FA * C  # free elems per partition per tile (2048)
DT = mybir.dt.float32


def build_nc(R: int):
    NT = R // TP
    nc = bacc.Bacc("TRN2", target_bir_lowering=False, debug=False,
                   num_devices=NCORES, enable_asserts=False)

    xs = nc.dram_tensor("xs", [RPC, R, C], DT, kind="ExternalInput")
    corrt = nc.dram_tensor("corrt", [C, RPC], DT, kind="ExternalInput")
    invct = nc.dram_tensor("invct", [C, RPC], DT, kind="ExternalInput")
    w1t = nc.dram_tensor("w1t", [C, H], DT, kind="ExternalInput")
    b1c = nc.dram_tensor("b1c", [H, 1], DT, kind="ExternalInput")
    w2t = nc.dram_tensor("w2t", [H, C], DT, kind="ExternalInput")
    b2x2 = nc.dram_tensor("b2x2", [C, 1], DT, kind="ExternalInput")
    out = nc.dram_tensor("out", [RPC, R, C], DT, kind="ExternalOutput")

    def dram_tile_ap(handle, r, t):
        return handle.ap()[r, t * TP:(t + 1) * TP, :].rearrange(
            "(p a) c -> p (a c)", p=128)

    with tile.TileContext(nc) as tc:
        with (
            tc.tile_pool(name="const", bufs=1) as const,
            tc.tile_pool(name="xpool", bufs=8) as xpool,
            tc.tile_pool(name="accs", bufs=1) as accs,
            tc.tile_pool(name="small", bufs=1) as small,
            tc.tile_pool(name="psum_t", bufs=2, space="PSUM") as psum_t,
            tc.tile_pool(name="psum_w", bufs=2, space="PSUM") as psum_w,
        ):
            # constants
            ident = const.tile([128, 128], DT)
            make_identity(nc, ident[:])
            ones_row = const.tile([1, 128], DT)
            nc.vector.memset(ones_row[:], 1.0)
            corrt_sb = const.tile([C, RPC], DT)
            nc.sync.dma_start(out=corrt_sb[:], in_=corrt.ap())
            invct_sb = const.tile([C, RPC], DT)
            nc.sync.dma_start(out=invct_sb[:], in_=invct.ap())
            w1t_sb = const.tile([C, H], DT)
            nc.sync.dma_start(out=w1t_sb[:], in_=w1t.ap())
            b1c_sb = const.tile([H, 1], DT)
            nc.sync.dma_start(out=b1c_sb[:], in_=b1c.ap())
            w2t_sb = const.tile([H, C], DT)
            nc.sync.dma_start(out=w2t_sb[:], in_=w2t.ap())
            b2x2_sb = const.tile([C, 1], DT)
            nc.sync.dma_start(out=b2x2_sb[:], in_=b2x2.ap())

            # phase 1: per-range running sum / max over streamed tiles
            rhs4 = small.tile([C, 2 * RPC], DT)  # cols: avg0, avg1, mx0, mx1
            for r in range(RPC):
                s_acc = accs.tile([128, F], DT, tag=f"s_acc{r}")
                m_acc = accs.tile([128, F], DT, tag=f"m_acc{r}")
                nc.vector.memset(s_acc[:], 0.0)
                nc.vector.memset(m_acc[:], -1e30)
                for t in range(NT):
                    xt = xpool.tile([128, F], DT, tag="xt")
                    nc.sync.dma_start(out=xt[:], in_=dram_tile_ap(xs, r, t))
                    nc.vector.tensor_add(s_acc[:], s_acc[:], xt[:])
                    nc.vector.tensor_max(m_acc[:], m_acc[:], xt[:])

                # fold free axis: [128, (a c)] -> [128, c]
                srow = small.tile([128, C], DT, tag=f"srow{r}")
                nc.vector.reduce_sum(
                    out=srow[:], in_=s_acc[:].rearrange("p (a c) -> p c a", c=C),
                    axis=mybir.AxisListType.X)
                mrow = small.tile([128, C], DT, tag=f"mrow{r}")
                nc.vector.reduce_max(
                    out=mrow[:], in_=m_acc[:].rearrange("p (a c) -> p c a", c=C),
                    axis=mybir.AxisListType.X)

                # fold partition axis: transpose [128, C] -> [C, 128], reduce
                srow_t = psum_t.tile([C, 128], DT, tag="tr")
                nc.tensor.transpose(out=srow_t[:], in_=srow[:], identity=ident[:])
                sum_col = small.tile([C, 1], DT, tag=f"sum_col{r}")
                nc.vector.reduce_sum(out=sum_col[:], in_=srow_t[:],
                                     axis=mybir.AxisListType.X)
                mrow_t = psum_t.tile([C, 128], DT, tag="tr")
                nc.tensor.transpose(out=mrow_t[:], in_=mrow[:], identity=ident[:])
                nc.vector.reduce_max(out=rhs4[:, RPC + r:RPC + r + 1], in_=mrow_t[:],
                                     axis=mybir.AxisListType.X)

                # avg = (sum - corr) * invc
                nc.vector.tensor_sub(sum_col[:], sum_col[:], corrt_sb[:, r:r + 1])
                nc.vector.tensor_mul(rhs4[:, r:r + 1], sum_col[:],
                                     invct_sb[:, r:r + 1])

            # tiny MLP: att = sigmoid(mlp(avg) + mlp(mx)); scale = 1 + att
            h_ps = psum_w.tile([H, 2 * RPC], DT, tag="mm")
            nc.tensor.matmul(out=h_ps[:], lhsT=w1t_sb[:], rhs=rhs4[:],
                             start=True, stop=True)
            h_sb = small.tile([H, 2 * RPC], DT)
            nc.scalar.activation(out=h_sb[:], in_=h_ps[:],
                                 func=mybir.ActivationFunctionType.Relu,
                                 bias=b1c_sb[:])
            z_ps = psum_w.tile([C, 2 * RPC], DT, tag="mm")
            nc.tensor.matmul(out=z_ps[:], lhsT=w2t_sb[:], rhs=h_sb[:],
                             start=True, stop=True)
            z_sb = small.tile([C, 2 * RPC], DT)
            nc.vector.tensor_copy(z_sb[:], z_ps[:])
            zsum = small.tile([C, RPC], DT)
            nc.vector.tensor_add(zsum[:], z_sb[:, 0:RPC], z_sb[:, RPC:2 * RPC])
            scale_t = small.tile([C, RPC], DT)
            nc.scalar.activation(out=scale_t[:], in_=zsum[:],
                                 func=mybir.ActivationFunctionType.Sigmoid,
                                 bias=b2x2_sb[:])
            nc.vector.tensor_scalar_add(scale_t[:], scale_t[:], 1.0)

            # broadcast each range's scale column to [128, C]
            mults = []
            for r in range(RPC):
                row_ps = psum_w.tile([1, C], DT, tag="row")
                nc.tensor.transpose(out=row_ps[:], in_=scale_t[:, r:r + 1],
                                    identity=ident[:C, :C])
                row_sb = small.tile([1, C], DT, tag=f"row_sb{r}")
                nc.vector.tensor_copy(row_sb[:], row_ps[:])
                bcast_ps = psum_w.tile([128, C], DT, tag="bc")
                nc.tensor.matmul(out=bcast_ps[:], lhsT=ones_row[:], rhs=row_sb[:],
                                 start=True, stop=True)
                mult = accs.tile([128, C], DT, tag=f"mult{r}")
                nc.vector.tensor_copy(mult[:], bcast_ps[:])
                mults.append(mult)

            # phase 2: out = x * scale[batch]
            for r in range(RPC):
                mult_bc = mults[r][:].unsqueeze(1).to_broadcast([128, FA, C])
                for t in range(NT):
                    xt = xpool.tile([128, FA, C], DT, tag="xt")
                    nc.sync.dma_start(out=xt[:], in_=dram_tile_ap(xs, r, t))
                    nc.vector.tensor_mul(xt[:], xt[:], mult_bc)
                    nc.sync.dma_start(out=dram_tile_ap(out, r, t), in_=xt[:])

    nc.compile()
    return nc


_CACHE: dict[int, object] = {}


def kernel(x, batch_ids, W1, b1, W2, b2):
    x = np.ascontiguousarray(np.asarray(x, dtype=np.float32))
    batch_ids = np.asarray(batch_ids, dtype=np.int32)
    W1 = np.asarray(W1, dtype=np.float32)
    b1 = np.asarray(b1, dtype=np.float32)
    W2 = np.asarray(W2, dtype=np.float32)
    b2 = np.asarray(b2, dtype=np.float32)

    N = x.shape[0]
    bounds = np.searchsorted(batch_ids, np.arange(B + 1), side="left")
    counts = np.diff(bounds)
    R = max(TP, int(-(-counts.max() // TP)) * TP)

    nc = _CACHE.get(R)
    if nc is None:
        nc = _CACHE[R] = build_nc(R)

    xp = np.empty((NCORES, RPC, R, C), np.float32)
    corrt = np.zeros((NCORES, C, RPC), np.float32)
    invct = np.zeros((NCORES, C, RPC), np.float32)
    for b in range(B):
        core, r = divmod(b, RPC)
        s, e = int(bounds[b]), int(bounds[b + 1])
        n = e - s
        xp[core, r, :n] = x[s:e]
        pad = x[s] if n > 0 else np.zeros(C, np.float32)
        xp[core, r, n:] = pad
        corrt[core, :, r] = np.float64(R - n) * pad.astype(np.float64)
        invct[core, :, r] = 1.0 / max(n, 1)

    w1t = np.ascontiguousarray(W1.T)  # [C, H]
    b1c = np.ascontiguousarray(b1.reshape(H, 1))
    w2t = np.ascontiguousarray(W2.T)  # [H, C]
    b2x2 = np.ascontiguousarray((2.0 * b2).reshape(C, 1))

    in_maps = [
        {
            "xs": xp[core],
            "corrt": np.ascontiguousarray(corrt[core]),
            "invct": np.ascontiguousarray(invct[core]),
            "w1t": w1t,
            "b1c": b1c,
            "w2t": w2t,
            "b2x2": b2x2,
        }
        for core in range(NCORES)
    ]

    res = run_bass_kernel_spmd(nc, in_maps, core_ids=list(range(NCORES)))

    out = np.empty((N, C), np.float32)
    for b in range(B):
        core, r = divmod(b, RPC)
        s, e = int(bounds[b]), int(bounds[b + 1])
        out[s:e] = res.results[core]["out"][r, : e - s]
    return out
